# revision 16
# baseline (speedup 1.0000x reference)
"""Per-sample grouped cross-correlation (SiamFC-style) on 8 trn2 NeuronCores.

out[b,0,i,j] = sum_{c,u,v} z[b,c,i+u,j+v] * x[b,c,u,v]
  x: [512, 256, 6, 6]  z: [512, 256, 22, 22]  out: [512, 1, 17, 17]

Data-parallel over batch: 64 samples per core. Per sample on-device:
  stage 1: A[uv=36, q=484] = x[c,36]^T @ z[c,484]  (2 K-tiles of 128, bf16,
           PSUM fp32), evicted CONTIGUOUSLY (bf16 cast) to A_cont[36, s, q].
  B-move : one SBUF->SBUF DMA per 16-sample wave re-partitions to
           B[s, uv*484+q]  (968-byte descriptors, samples on partitions).
  stage 2: out[s, i, j] = sum_uv B[s, uv*484 + (i+u)*22 + (j+v)] -- the
           window offset is constant across partitions, so this is 36
           aligned tensor_adds split across vector+gpsimd. No gather DMAs,
           no stage-2 matmuls, no host de-transpose.
"""

import numpy as np
import ml_dtypes

import concourse.bass as bass
import concourse.mybir as mybir
from concourse import tile
from concourse.tile import TileContext
from concourse.vector_clock import ScopedClock
from concourse.bass_utils import run_bass_kernel_spmd

N_CORES = 8
B, C, KX, KZ, KO = 512, 256, 6, 22, 17
SPC = B // N_CORES          # 64 samples per core
NQ = KZ * KZ                # 484
NO = KO * KO                # 289
NUV = KX * KX               # 36
SBLK = 8                    # z streaming block (samples)
WAVE = 16                   # B-move granularity (samples)
NW = SPC // WAVE            # 4 waves

BF16 = mybir.dt.bfloat16
F32 = mybir.dt.float32


def _patched_drain_and_barrier(self, tick_clock, wait_clock):
    # walrus in this env rejects instructions carrying >2 sync waits; split
    # the kernel-tail drain waits across standalone nops (<=1 wait each).
    nc = self.nc
    drain_inst = nc.sync.drain()
    wait_clock.add_sem_waits(drain_inst.ins, ScopedClock({None: tick_clock.global_clock}))
    inst = drain_inst.ins
    si = inst.sync_info
    ow = list(si.on_wait) if si is not None and si.on_wait else []
    if len(ow) > 1:
        si.on_wait = ow[:1]
        for w in ow[1:]:
            n = nc.sync.nop()
            ni = n.ins
            if ni.sync_info is None:
                ni.sync_info = mybir.SyncInfo(on_wait=[w], on_update=[])
            else:
                ni.sync_info.on_wait = [w]
    nc.all_engine_barrier()
    popped = nc._tile_sem_poison_stack.pop()
    assert popped is self._sem_poison
    nc.clear_and_free_semaphores(list(self.sems.allocated().values()))
    nc.all_engine_barrier()


tile.TileContext._drain_and_barrier = _patched_drain_and_barrier

MAX_WAITS = 1  # walrus in this env rejects multi-wait instructions


def _split_excess_waits(nc):
    """Hoist excess sem waits onto same-engine NOPs inserted just before."""
    engs = {
        mybir.EngineType.PE: nc.tensor,
        mybir.EngineType.DVE: nc.vector,
        mybir.EngineType.Activation: nc.scalar,
        mybir.EngineType.Pool: nc.gpsimd,
        mybir.EngineType.SP: nc.sync,
    }
    cur_il = nc.cur_bb.bb.instructions  # nop() appends here
    for bname, bassbb in list(nc.bb_map.items()):
        bb = bassbb.bb
        il = list(bb.instructions)
        out = []
        changed = False
        for inst in il:
            si = inst.sync_info
            ow = list(si.on_wait) if (si is not None and si.on_wait) else []
            if len(ow) > MAX_WAITS and inst.engine in engs:
                changed = True
                extra, keep = ow[:-MAX_WAITS], ow[-MAX_WAITS:]
                for k in range(0, len(extra), MAX_WAITS):
                    n = engs[inst.engine].nop()
                    ni = n.ins
                    popped = cur_il.pop()
                    assert popped.name == ni.name
                    ni.sync_info = mybir.SyncInfo(
                        on_wait=extra[k:k + MAX_WAITS], on_update=[])
                    out.append(ni)
                si.on_wait = keep
            out.append(inst)
        if changed:
            bb.instructions = out


def build_nc():
    nc = bass.Bass("TRN2", target_bir_lowering=False, debug=False, num_devices=N_CORES)
    # host-prepared layouts (c = h*128 + p):
    z_in = nc.declare_dram_parameter("z", [128, 2, SPC, NQ], BF16, isOutput=False)
    x_in = nc.declare_dram_parameter("x", [128, 2, SPC, NUV], BF16, isOutput=False)
    y_out = nc.declare_dram_parameter("y", [SPC, NO], F32, isOutput=True)

    from contextlib import ExitStack

    with TileContext(nc) as tc, ExitStack() as ctx:
        xpool = ctx.enter_context(tc.tile_pool(name="xp", bufs=4))
        zpool = ctx.enter_context(tc.tile_pool(name="zp", bufs=3))
        apool = ctx.enter_context(tc.tile_pool(name="ap", bufs=2))
        bpool = ctx.enter_context(tc.tile_pool(name="bp", bufs=1))
        opool = ctx.enter_context(tc.tile_pool(name="op", bufs=2))
        pspool = ctx.enter_context(tc.tile_pool(name="ps", bufs=6, space="PSUM"))

        # per-wave x tiles so the first matmul doesn't wait on the full x load
        x_tiles = []
        for w in range(NW):
            x_w = xpool.tile([128, 2, WAVE, NUV], BF16)
            nc.sync.dma_start(
                out=x_w[:], in_=x_in[:, :, w * WAVE:(w + 1) * WAVE, :])
            x_tiles.append(x_w)

        # two sample-halves so the first half's stage 2 overlaps waves 2-3;
        # padded by KX-1 so the last uv window's (17, 22)-shaped AP stays
        # in-bounds (the overhang is never read due to the [:, :, :KO] crop)
        HALF = SPC // 2
        B_halves = [bpool.tile([HALF, NUV * NQ + KX - 1], BF16, name=f"Bh{i}")
                    for i in range(2)]

        def stage2(half):
            Bq = B_halves[half][:]

            def win(uv):
                # full 374-elem contiguous slice: rows are 22 wide, the extra
                # 5 columns are accumulated too and cropped at the combine
                u, v = uv // KX, uv % KX
                off = uv * NQ + u * KZ + v
                return Bq[:, off:off + KO * KZ]

            # bf16 partial accumulators, contiguous adds; final crop+combine f32
            acc_v = opool.tile([HALF, KO * KZ], BF16)
            acc_g = opool.tile([HALF, KO * KZ], BF16)
            acc_f = opool.tile([HALF, KO, KO], F32)
            nc.vector.tensor_add(out=acc_v[:], in0=win(0), in1=win(1))
            nc.gpsimd.tensor_add(out=acc_g[:], in0=win(18), in1=win(19))
            for uv in range(2, 18):
                nc.vector.tensor_add(out=acc_v[:], in0=acc_v[:], in1=win(uv))
            for uv in range(20, 36):
                nc.gpsimd.tensor_add(out=acc_g[:], in0=acc_g[:], in1=win(uv))
            crop_v = acc_v[:].rearrange("s (i j) -> s i j", i=KO, j=KZ)[:, :, :KO]
            crop_g = acc_g[:].rearrange("s (i j) -> s i j", i=KO, j=KZ)[:, :, :KO]
            nc.vector.tensor_add(out=acc_f[:], in0=crop_v, in1=crop_g)
            nc.sync.dma_start(
                out=y_out[half * HALF:(half + 1) * HALF, :],
                in_=acc_f[:].rearrange("s i j -> s (i j)"))

        for w in range(NW):
            A_cont = apool.tile([NUV, WAVE, NQ], BF16)
            # first-ever block is small so the pipeline starts sooner
            blocks = [2, 6, 8] if w == 0 else [8, 8]
            sw = 0
            for nblk in blocks:
                z_sb = zpool.tile([128, 2, nblk, NQ], BF16)
                s0 = w * WAVE + sw
                nc.sync.dma_start(out=z_sb[:], in_=z_in[:, :, s0:s0 + nblk, :])
                for sl in range(nblk):
                    aps = pspool.tile([NUV, NQ], F32)
                    for h in range(2):
                        nc.tensor.matmul(
                            aps[:],
                            lhsT=x_tiles[w][:, h, sw, :],
                            rhs=z_sb[:, h, sl, :],
                            start=(h == 0),
                            stop=(h == 1),
                        )
                    # contiguous PSUM -> SBUF eviction (bf16 downcast), then
                    # re-partition this sample with one direct SBUF->SBUF DMA
                    # (partition-outer src -> single-partition row; this AP
                    # shape lowers correctly, unlike the mid-AP partition dim).
                    # Even samples: scalar evicts and triggers (no stall);
                    # odd: vector evicts, sync triggers.
                    row = (w % 2) * WAVE + sw
                    brow = B_halves[w // 2][row:row + 1, 0:NUV * NQ]
                    if sw % 2 == 0:
                        nc.scalar.copy(out=A_cont[:, sw, :], in_=aps[:])
                        nc.scalar.dma_start(out=brow, in_=A_cont[:, sw, :])
                    else:
                        nc.vector.tensor_copy(A_cont[:, sw, :], aps[:])
                        nc.sync.dma_start(out=brow, in_=A_cont[:, sw, :])
                    sw += 1
            if w == 1:
                stage2(0)
        stage2(1)

    _split_excess_waits(nc)
    return nc


_NC_CACHE = None


def _get_nc():
    global _NC_CACHE
    if _NC_CACHE is None:
        _NC_CACHE = build_nc()
    return _NC_CACHE


def _prep_inputs(x: np.ndarray, z: np.ndarray):
    # z: [512, 256, 22, 22] -> [8, 128, 2, 64, 484] bf16 (c = h*128 + p)
    zz = np.ascontiguousarray(z, dtype=np.float32).reshape(N_CORES, SPC, 2, 128, NQ)
    zz = zz.transpose(0, 3, 2, 1, 4).astype(ml_dtypes.bfloat16)
    xx = np.ascontiguousarray(x, dtype=np.float32).reshape(N_CORES, SPC, 2, 128, NUV)
    xx = xx.transpose(0, 3, 2, 1, 4).astype(ml_dtypes.bfloat16)
    return xx, zz


def kernel(x: np.ndarray, z: np.ndarray, trace: bool = False):
    nc = _get_nc()
    xx, zz = _prep_inputs(x, z)
    in_maps = [{"x": np.ascontiguousarray(xx[i]), "z": np.ascontiguousarray(zz[i])}
               for i in range(N_CORES)]
    res = run_bass_kernel_spmd(nc, in_maps, list(range(N_CORES)), trace=trace)
    ys = [res.results[i]["y"].reshape(SPC, KO, KO) for i in range(N_CORES)]
    out = np.concatenate(ys, axis=0).reshape(B, 1, KO, KO).astype(np.float32)
    if trace:
        return out, res
    return out
